# revision 20
# baseline (speedup 1.0000x reference)
"""Fused pairwise-MLP kernel for Trainium2 (8 NeuronCores, SPMD data-parallel).

Computes log_q[i, j] = W3 @ gelu(W2 @ gelu(a[j] + b[i] + b1) + b2) + b3
with a = z1 @ W1a.T, b = z2 @ W1b.T  (W1 = [W1a | W1b]), N=1024, H=EMB=128.

Sharding: rows of i (z2) split across 8 cores, z1 + weights replicated
(host-side sharding; no collectives). The [N, N, H] intermediates are never
materialized in HBM — everything stays in SBUF/PSUM per 128-row i-tile.
Per core: for each of its 128 i values, one ACT gelu over [128, 1024],
two W2 matmuls, a second gelu, and an M=1 W3-dot matmul pair.
"""

import numpy as np

import concourse.bacc as bacc
import concourse.bass as bass
import concourse.tile as tile
import concourse.mybir as mybir
from concourse import bass_utils
from concourse.masks import make_identity

N = 1024
EMB = 128
HID = 128
NCORES = 8
SH = N // NCORES  # i-rows per core
F32 = mybir.dt.float32
F32R = mybir.dt.float32r  # fp32 bits, single-pass reduced-precision matmul
BF16 = mybir.dt.bfloat16
GELU = mybir.ActivationFunctionType.Gelu

# W3-dot implementation: "f32r" = two serial M=1 fp32r matmuls per row;
# "bf16ct" = bf16 4-way column-tiled (4 rows concurrently in the PE array).
MM2_MODE = "bf16ct"


def _build():
    nc = bacc.Bacc("TRN2", target_bir_lowering=False, debug=False)

    z1_d = nc.dram_tensor("z1", (N, EMB), F32, kind="ExternalInput")
    z2_d = nc.dram_tensor("z2s", (SH, EMB), F32, kind="ExternalInput")
    w1_d = nc.dram_tensor("W1", (HID, 2 * EMB), F32, kind="ExternalInput")
    b1_d = nc.dram_tensor("b1", (HID,), F32, kind="ExternalInput")
    w2_d = nc.dram_tensor("W2", (HID, HID), F32, kind="ExternalInput")
    b2_d = nc.dram_tensor("b2", (HID,), F32, kind="ExternalInput")
    w3_d = nc.dram_tensor("W3", (1, HID), F32, kind="ExternalInput")
    b3_d = nc.dram_tensor("b3", (1,), F32, kind="ExternalInput")
    out_d = nc.dram_tensor("out", (SH, N), F32, kind="ExternalOutput")

    with tile.TileContext(nc) as tc:
        _body(tc, out_d, z1_d, z2_d, w1_d, b1_d, w2_d, b2_d, w3_d, b3_d)

    nc.compile()
    return nc


def _body(tc, out_d, z1_d, z2_d, w1_d, b1_d, w2_d, b2_d, w3_d, b3_d):
    nc = tc.nc
    with (
        tc.tile_pool(name="const", bufs=1) as const,
        tc.tile_pool(name="zload", bufs=3) as zload,
        tc.tile_pool(name="h1p", bufs=4) as h1p,
        tc.tile_pool(name="h2p", bufs=6) as h2p,
        tc.tile_pool(name="srows", bufs=6) as srows,
        tc.tile_pool(name="ps", bufs=2, space="PSUM") as ps,
        tc.tile_pool(name="rowp", bufs=1, space="PSUM") as rowp,
    ):
        ident = const.tile([128, 128], F32)
        make_identity(nc, ident)

        # ---- load weights/biases ----
        w1_sb = const.tile([128, 2 * EMB], F32)
        nc.sync.dma_start(out=w1_sb, in_=w1_d.ap())
        w2_sb = const.tile([128, HID], F32)
        nc.sync.dma_start(out=w2_sb, in_=w2_d.ap())
        w3row_sb = const.tile([1, HID], F32)
        nc.sync.dma_start(out=w3row_sb, in_=w3_d.ap())
        b1_sb = const.tile([128, 1], F32)
        nc.sync.dma_start(out=b1_sb, in_=b1_d.ap().rearrange("(p o) -> p o", o=1))
        b2_sb = const.tile([128, 1], F32)
        nc.sync.dma_start(out=b2_sb, in_=b2_d.ap().rearrange("(p o) -> p o", o=1))
        b3_sb = const.tile([1, 1], F32)
        nc.sync.dma_start(out=b3_sb, in_=b3_d.ap().rearrange("(p o) -> p o", o=1))

        # ---- transpose z1 -> z1T [emb, N], z2 shard -> z2T [emb, SH] ----
        z1T_sb = const.tile([128, N], F32)
        for t in range(N // 128):
            zt = zload.tile([128, 128], F32, tag="zt")
            nc.sync.dma_start(out=zt, in_=z1_d.ap()[t * 128 : (t + 1) * 128, :])
            tp = ps.tile([128, 1024], F32, tag="ps1")
            nc.tensor.transpose(tp[:, 0:128], zt, ident)
            nc.vector.tensor_copy(z1T_sb[:, t * 128 : (t + 1) * 128], tp[:, 0:128])

        z2_sb = zload.tile([128, EMB], F32, tag="zt")
        nc.sync.dma_start(out=z2_sb, in_=z2_d.ap())
        z2T_sb = const.tile([128, SH], F32)
        tp = ps.tile([128, 1024], F32, tag="ps1")
        nc.tensor.transpose(tp[:, 0:128], z2_sb, ident)
        nc.vector.tensor_copy(z2T_sb, tp[:, 0:128])

        # ---- transpose weights: W1a.T, W1b.T, W2.T, W3.T (as matmul lhsT) ----
        w1aT_sb = const.tile([128, 128], F32)
        w1bT_sb = const.tile([128, 128], F32)
        w2T_sb = const.tile([128, 128], F32R)
        for src, dst in (
            (w1_sb[:, 0:EMB], w1aT_sb),
            (w1_sb[:, EMB : 2 * EMB], w1bT_sb),
            (w2_sb, w2T_sb),
        ):
            tp = ps.tile([128, 1024], F32, tag="ps1")
            nc.tensor.transpose(tp[:, 0:128], src, ident)
            nc.vector.tensor_copy(dst, tp[:, 0:128])

        # W3.T as matmul lhsT. For bf16ct: padded to [128, 32] (cols 1-31
        # zero) so the W3-dot matmuls are valid M=32 column tiles; only row 0
        # of each 32-row block is real.
        tp = ps.tile([128, 1024], F32, tag="ps1")
        nc.tensor.transpose(tp[:, 0:1], w3row_sb, ident[0:1, 0:1])
        if MM2_MODE == "bf16ct":
            w3pad_f = const.tile([128, 32], F32)
            nc.vector.memset(w3pad_f, 0.0)
            nc.vector.tensor_copy(w3pad_f[:, 0:1], tp[:, 0:1])
            w3T_sb = const.tile([128, 32], BF16)
            nc.vector.tensor_copy(w3T_sb, w3pad_f)
        else:
            w3T_sb = const.tile([128, 1], F32R)
            nc.vector.tensor_copy(w3T_sb, tp[:, 0:1])

        # ---- a[h, j] for all j; b_pp[h, i] = b[h, i] + b1 for my i-shard ----
        tpa = ps.tile([128, 1024], F32, tag="ps1")
        nc.tensor.matmul(tpa[:, 0:512], w1aT_sb, z1T_sb[:, 0:512])
        nc.tensor.matmul(tpa[:, 512:1024], w1aT_sb, z1T_sb[:, 512:1024])
        a_sb = const.tile([128, N], F32)
        nc.vector.tensor_copy(a_sb, tpa)

        tpb = ps.tile([128, 1024], F32, tag="ps1")
        nc.tensor.matmul(tpb[:, 0:SH], w1bT_sb, z2T_sb)
        b_pp_sb = const.tile([128, SH], F32)
        nc.vector.tensor_scalar_add(b_pp_sb, tpb[:, 0:SH], b1_sb[:, 0:1])

        # ---- main loop over my 128 i values, in quads of 4 ----
        # ACT does one 4096-wide gelu1 per quad (the per-i bias adds
        # a + b_pp[:, i] are precomputed into `pre` by DVE/GPSIMD, 2+2 per
        # quad) and one 1024-wide gelu2 per i. Quads are emitted 2 ahead so
        # ACT never waits on the W2 matmuls. The W3-dot runs as 4 concurrent
        # bf16 M=32 column-tile matmuls (outputs at PSUM partitions
        # 0/32/64/96), one pair of matmul groups per quad.
        NQ = SH // 4
        h1qs = [None] * NQ

        def emit_quad_g1(q):
            pre = h1p.tile([128, 4 * N], F32, tag="pre", name="pre", bufs=2)
            for k in range(4):
                i = 4 * q + k
                eng = nc.vector if k < 2 else nc.gpsimd
                eng.tensor_scalar_add(
                    pre[:, k * N : (k + 1) * N], a_sb, b_pp_sb[:, i : i + 1]
                )
            h1q = h1p.tile([128, 4 * N], F32R, tag="h1q", name="h1q", bufs=3)
            nc.scalar.activation(h1q, pre, GELU)
            h1qs[q] = h1q

        emit_quad_g1(0)
        emit_quad_g1(1)
        h2s = [None] * 4
        for q in range(NQ):
            if q + 2 < NQ:
                emit_quad_g1(q + 2)
            h1q = h1qs[q]
            h1qs[q] = None

            for k in range(4):
                base = k * N
                ps1 = ps.tile([128, N], F32, tag="ps1")
                nc.tensor.matmul(ps1[:, 0:512], w2T_sb, h1q[:, base : base + 512])
                nc.tensor.matmul(
                    ps1[:, 512:1024], w2T_sb, h1q[:, base + 512 : base + 1024]
                )
                h2 = h2p.tile([128, N], BF16, tag="h2")
                nc.scalar.activation(h2, ps1, GELU, bias=b2_sb[:, 0:1])
                h2s[k] = h2

            pr = rowp.tile([128, N], F32, tag="row")
            for k in range(4):
                nc.tensor.matmul(
                    pr[32 * k : 32 * k + 32, 0:512],
                    w3T_sb,
                    h2s[k][:, 0:512],
                    tile_position=(0, 32 * k),
                )
                nc.tensor.matmul(
                    pr[32 * k : 32 * k + 32, 512:1024],
                    w3T_sb,
                    h2s[k][:, 512:1024],
                    tile_position=(0, 32 * k),
                )
            for k in range(4):
                ii = 4 * q + k
                srow = srows.tile([1, N], F32, tag="srow")
                nc.vector.tensor_scalar_add(
                    srow, pr[32 * k : 32 * k + 1, :], b3_sb[0:1, 0:1]
                )
                nc.sync.dma_start(out=out_d.ap()[ii : ii + 1, :], in_=srow)


_NC_CACHE = None


def kernel(z1, z2, W1, b1, W2, b2, W3, b3):
    global _NC_CACHE
    if _NC_CACHE is None:
        _NC_CACHE = _build()
    nc = _NC_CACHE

    z1 = np.ascontiguousarray(np.asarray(z1, dtype=np.float32))
    z2 = np.ascontiguousarray(np.asarray(z2, dtype=np.float32))
    W1 = np.ascontiguousarray(np.asarray(W1, dtype=np.float32))
    b1 = np.ascontiguousarray(np.asarray(b1, dtype=np.float32))
    W2 = np.ascontiguousarray(np.asarray(W2, dtype=np.float32))
    b2 = np.ascontiguousarray(np.asarray(b2, dtype=np.float32))
    W3 = np.ascontiguousarray(np.asarray(W3, dtype=np.float32))
    b3 = np.ascontiguousarray(np.asarray(b3, dtype=np.float32))

    in_maps = [
        {
            "z1": z1,
            "z2s": np.ascontiguousarray(z2[c * SH : (c + 1) * SH]),
            "W1": W1,
            "b1": b1,
            "W2": W2,
            "b2": b2,
            "W3": W3,
            "b3": b3,
        }
        for c in range(NCORES)
    ]
    res = bass_utils.run_bass_kernel_spmd(nc, in_maps, core_ids=list(range(NCORES)))
    return np.concatenate([r["out"] for r in res.results], axis=0)


if __name__ == "__main__":
    rng = np.random.default_rng(0)
    s1 = 1.0 / np.sqrt(2 * EMB)
    s2 = 1.0 / np.sqrt(HID)
    ins = dict(
        z1=rng.standard_normal((N, EMB), dtype=np.float32),
        z2=rng.standard_normal((N, EMB), dtype=np.float32),
        W1=rng.uniform(-s1, s1, (HID, 2 * EMB)).astype(np.float32),
        b1=rng.uniform(-s1, s1, (HID,)).astype(np.float32),
        W2=rng.uniform(-s2, s2, (HID, HID)).astype(np.float32),
        b2=rng.uniform(-s2, s2, (HID,)).astype(np.float32),
        W3=rng.uniform(-s2, s2, (1, HID)).astype(np.float32),
        b3=rng.uniform(-s2, s2, (1,)).astype(np.float32),
    )
    out = kernel(**ins)
    print("out", out.shape, out.dtype, out[:2, :4])


# revision 23
# speedup vs baseline: 3.3345x; 3.3345x over previous
"""Fused pairwise-MLP kernel for Trainium2 (8 NeuronCores, SPMD data-parallel).

Computes log_q[i, j] = W3 @ gelu(W2 @ gelu(a[j] + b[i] + b1) + b2) + b3
with a = z1 @ W1a.T, b = z2 @ W1b.T  (W1 = [W1a | W1b]), N=1024, H=EMB=128.

Sharding: rows of i (z2) split across 8 cores, z1 + weights replicated
(host-side sharding; no collectives). The [N, N, H] intermediates are never
materialized in HBM — everything stays in SBUF/PSUM per 128-row i-tile.
Per core: for each of its 128 i values, one ACT gelu over [128, 1024],
two W2 matmuls, a second gelu, and an M=1 W3-dot matmul pair.
"""

import numpy as np

import concourse.bacc as bacc
import concourse.bass as bass
import concourse.tile as tile
import concourse.mybir as mybir
from concourse import bass_utils
from concourse.masks import make_identity

N = 1024
EMB = 128
HID = 128
NCORES = 8
SH = N // NCORES  # i-rows per core
F32 = mybir.dt.float32
F32R = mybir.dt.float32r  # fp32 bits, single-pass reduced-precision matmul
BF16 = mybir.dt.bfloat16
GELU = mybir.ActivationFunctionType.Gelu

# W3-dot implementation: "f32r" = two serial M=1 fp32r matmuls per row;
# "bf16ct" = bf16 4-way column-tiled (4 rows concurrently in the PE array).
MM2_MODE = "bf16ct"


def _build():
    nc = bacc.Bacc("TRN2", target_bir_lowering=False, debug=False)

    z1_d = nc.dram_tensor("z1", (N, EMB), F32, kind="ExternalInput")
    z2_d = nc.dram_tensor("z2s", (SH, EMB), F32, kind="ExternalInput")
    w1_d = nc.dram_tensor("W1", (HID, 2 * EMB), F32, kind="ExternalInput")
    b1_d = nc.dram_tensor("b1", (HID,), F32, kind="ExternalInput")
    w2_d = nc.dram_tensor("W2", (HID, HID), F32, kind="ExternalInput")
    b2_d = nc.dram_tensor("b2", (HID,), F32, kind="ExternalInput")
    w3_d = nc.dram_tensor("W3", (1, HID), F32, kind="ExternalInput")
    b3_d = nc.dram_tensor("b3", (1,), F32, kind="ExternalInput")
    out_d = nc.dram_tensor("out", (SH, N), F32, kind="ExternalOutput")

    with tile.TileContext(nc) as tc:
        _body(tc, out_d, z1_d, z2_d, w1_d, b1_d, w2_d, b2_d, w3_d, b3_d)

    nc.compile()
    return nc


def _body(tc, out_d, z1_d, z2_d, w1_d, b1_d, w2_d, b2_d, w3_d, b3_d):
    nc = tc.nc
    with (
        tc.tile_pool(name="const", bufs=1) as const,
        tc.tile_pool(name="zload", bufs=3) as zload,
        tc.tile_pool(name="h1p", bufs=4) as h1p,
        tc.tile_pool(name="h2p", bufs=6) as h2p,
        tc.tile_pool(name="srows", bufs=6) as srows,
        tc.tile_pool(name="ps", bufs=2, space="PSUM") as ps,
        tc.tile_pool(name="apz", bufs=1, space="PSUM") as apz,
        tc.tile_pool(name="rowp", bufs=1, space="PSUM") as rowp,
    ):
        ident = const.tile([128, 128], F32)
        make_identity(nc, ident)

        # ---- load weights/biases ----
        w1_sb = const.tile([128, 2 * EMB], F32)
        nc.sync.dma_start(out=w1_sb, in_=w1_d.ap())
        w2_sb = const.tile([128, HID], F32)
        nc.sync.dma_start(out=w2_sb, in_=w2_d.ap())
        w3row_sb = const.tile([1, HID], F32)
        nc.sync.dma_start(out=w3row_sb, in_=w3_d.ap())
        b1_sb = const.tile([128, 1], F32)
        nc.sync.dma_start(out=b1_sb, in_=b1_d.ap().rearrange("(p o) -> p o", o=1))
        b2_sb = const.tile([128, 1], F32)
        nc.sync.dma_start(out=b2_sb, in_=b2_d.ap().rearrange("(p o) -> p o", o=1))
        b3_sb = const.tile([1, 1], F32)
        nc.sync.dma_start(out=b3_sb, in_=b3_d.ap().rearrange("(p o) -> p o", o=1))

        # ---- transpose z1 -> z1T [emb, N], z2 shard -> z2T [emb, SH] ----
        z1T_sb = const.tile([128, N], F32)
        for t in range(N // 128):
            zt = zload.tile([128, 128], F32, tag="zt")
            nc.sync.dma_start(out=zt, in_=z1_d.ap()[t * 128 : (t + 1) * 128, :])
            tp = ps.tile([128, 1024], F32, tag="ps1")
            nc.tensor.transpose(tp[:, 0:128], zt, ident)
            nc.vector.tensor_copy(z1T_sb[:, t * 128 : (t + 1) * 128], tp[:, 0:128])

        z2_sb = zload.tile([128, EMB], F32, tag="zt")
        nc.sync.dma_start(out=z2_sb, in_=z2_d.ap())
        z2T_sb = const.tile([128, SH], F32)
        tp = ps.tile([128, 1024], F32, tag="ps1")
        nc.tensor.transpose(tp[:, 0:128], z2_sb, ident)
        nc.vector.tensor_copy(z2T_sb, tp[:, 0:128])

        # ---- transpose weights: W1a.T, W1b.T, W2.T, W3.T (as matmul lhsT) ----
        w1aT_sb = const.tile([128, 128], F32)
        w1bT_sb = const.tile([128, 128], F32)
        w2T_sb = const.tile([128, 128], F32R)
        for src, dst in (
            (w1_sb[:, 0:EMB], w1aT_sb),
            (w1_sb[:, EMB : 2 * EMB], w1bT_sb),
            (w2_sb, w2T_sb),
        ):
            tp = ps.tile([128, 1024], F32, tag="ps1")
            nc.tensor.transpose(tp[:, 0:128], src, ident)
            nc.vector.tensor_copy(dst, tp[:, 0:128])

        # W3.T as matmul lhsT. For bf16ct: padded to [128, 32] (cols 1-31
        # zero) so the W3-dot matmuls are valid M=32 column tiles; only row 0
        # of each 32-row block is real.
        tp = ps.tile([128, 1024], F32, tag="ps1")
        nc.tensor.transpose(tp[:, 0:1], w3row_sb, ident[0:1, 0:1])
        if MM2_MODE == "bf16ct":
            w3pad_f = const.tile([128, 32], F32)
            nc.vector.memset(w3pad_f, 0.0)
            nc.vector.tensor_copy(w3pad_f[:, 0:1], tp[:, 0:1])
            w3T_sb = const.tile([128, 32], BF16)
            nc.vector.tensor_copy(w3T_sb, w3pad_f)
        else:
            w3T_sb = const.tile([128, 1], F32R)
            nc.vector.tensor_copy(w3T_sb, tp[:, 0:1])

        # ---- a[h, j] for all j; b_pp[h, i] = b[h, i] + b1 for my i-shard ----
        a_ps = apz.tile([128, N], F32)  # persistent PSUM: gelu1 streams from here
        nc.tensor.matmul(a_ps[:, 0:512], w1aT_sb, z1T_sb[:, 0:512])
        nc.tensor.matmul(a_ps[:, 512:1024], w1aT_sb, z1T_sb[:, 512:1024])

        tpb = ps.tile([128, 1024], F32, tag="ps1")
        nc.tensor.matmul(tpb[:, 0:SH], w1bT_sb, z2T_sb)
        b_pp_sb = const.tile([128, SH], F32)
        nc.vector.tensor_scalar_add(b_pp_sb, tpb[:, 0:SH], b1_sb[:, 0:1])

        # ---- main loop over my 128 i values ----
        # Software-pipelined emission: gelu1 (per-i bias from the ACT bias
        # port) runs 2 iterations ahead so the ACT stream never stalls on the
        # W2 matmuls. The W3-dot runs as 4 concurrent bf16 M=32 column-tile
        # matmuls (outputs at PSUM partitions 0/32/64/96) per 4 i's.
        h1s = [None] * SH

        def emit_g1(i):
            h1s[i] = h1p.tile([128, N], F32R, tag="h1", name="h1")
            nc.scalar.activation(h1s[i], a_ps, GELU, bias=b_pp_sb[:, i : i + 1])

        emit_g1(0)
        emit_g1(1)
        h2s = [None] * 4
        for i in range(SH):
            if i + 2 < SH:
                emit_g1(i + 2)

            ps1 = ps.tile([128, N], F32, tag="ps1")
            nc.tensor.matmul(ps1[:, 0:512], w2T_sb, h1s[i][:, 0:512])
            nc.tensor.matmul(ps1[:, 512:1024], w2T_sb, h1s[i][:, 512:1024])
            h1s[i] = None

            h2 = h2p.tile([128, N], BF16, tag="h2")
            nc.scalar.activation(h2, ps1, GELU, bias=b2_sb[:, 0:1])
            h2s[i % 4] = h2

            if i % 4 == 3:
                pr = rowp.tile([128, N], F32, tag="row")
                for k in range(4):
                    nc.tensor.matmul(
                        pr[32 * k : 32 * k + 32, 0:512],
                        w3T_sb,
                        h2s[k][:, 0:512],
                        tile_position=(0, 32 * k),
                    )
                    nc.tensor.matmul(
                        pr[32 * k : 32 * k + 32, 512:1024],
                        w3T_sb,
                        h2s[k][:, 512:1024],
                        tile_position=(0, 32 * k),
                    )
                for k in range(4):
                    ii = i - 3 + k
                    srow = srows.tile([1, N], F32, tag="srow")
                    nc.vector.tensor_scalar_add(
                        srow, pr[32 * k : 32 * k + 1, :], b3_sb[0:1, 0:1]
                    )
                    nc.sync.dma_start(out=out_d.ap()[ii : ii + 1, :], in_=srow)


_NC_CACHE = None


def kernel(z1, z2, W1, b1, W2, b2, W3, b3):
    global _NC_CACHE
    if _NC_CACHE is None:
        _NC_CACHE = _build()
    nc = _NC_CACHE

    z1 = np.ascontiguousarray(np.asarray(z1, dtype=np.float32))
    z2 = np.ascontiguousarray(np.asarray(z2, dtype=np.float32))
    W1 = np.ascontiguousarray(np.asarray(W1, dtype=np.float32))
    b1 = np.ascontiguousarray(np.asarray(b1, dtype=np.float32))
    W2 = np.ascontiguousarray(np.asarray(W2, dtype=np.float32))
    b2 = np.ascontiguousarray(np.asarray(b2, dtype=np.float32))
    W3 = np.ascontiguousarray(np.asarray(W3, dtype=np.float32))
    b3 = np.ascontiguousarray(np.asarray(b3, dtype=np.float32))

    in_maps = [
        {
            "z1": z1,
            "z2s": np.ascontiguousarray(z2[c * SH : (c + 1) * SH]),
            "W1": W1,
            "b1": b1,
            "W2": W2,
            "b2": b2,
            "W3": W3,
            "b3": b3,
        }
        for c in range(NCORES)
    ]
    res = bass_utils.run_bass_kernel_spmd(nc, in_maps, core_ids=list(range(NCORES)))
    return np.concatenate([r["out"] for r in res.results], axis=0)


if __name__ == "__main__":
    rng = np.random.default_rng(0)
    s1 = 1.0 / np.sqrt(2 * EMB)
    s2 = 1.0 / np.sqrt(HID)
    ins = dict(
        z1=rng.standard_normal((N, EMB), dtype=np.float32),
        z2=rng.standard_normal((N, EMB), dtype=np.float32),
        W1=rng.uniform(-s1, s1, (HID, 2 * EMB)).astype(np.float32),
        b1=rng.uniform(-s1, s1, (HID,)).astype(np.float32),
        W2=rng.uniform(-s2, s2, (HID, HID)).astype(np.float32),
        b2=rng.uniform(-s2, s2, (HID,)).astype(np.float32),
        W3=rng.uniform(-s2, s2, (1, HID)).astype(np.float32),
        b3=rng.uniform(-s2, s2, (1,)).astype(np.float32),
    )
    out = kernel(**ins)
    print("out", out.shape, out.dtype, out[:2, :4])


# revision 26
# speedup vs baseline: 3.4880x; 1.0460x over previous
"""Fused pairwise-MLP kernel for Trainium2 (8 NeuronCores, SPMD data-parallel).

Computes log_q[i, j] = W3 @ gelu(W2 @ gelu(a[j] + b[i] + b1) + b2) + b3
with a = z1 @ W1a.T, b = z2 @ W1b.T  (W1 = [W1a | W1b]), N=1024, H=EMB=128.

Sharding: rows of i (z2) split across 8 cores, z1 + weights replicated
(host-side sharding; no collectives). The [N, N, H] intermediates are never
materialized in HBM — everything stays in SBUF/PSUM per 128-row i-tile.

The host only relays/relabels inputs (transposes, sharding, zero-padding W3);
all math runs on device. Per core and per i: one 1024-wide gelu on ACT
(bias port adds b[i]+b1), two fp32r W2 matmuls, a second gelu, and the W3
dot as bf16 M=32 column-tile matmuls batched 4-concurrent per quad of i's.
The kernel is ACT-bound: 2 gelu passes over 16.8M elements per core at
1 elem/lane/cycle @ 1.2 GHz.
"""

import numpy as np

import concourse.bacc as bacc
import concourse.bass as bass
import concourse.tile as tile
import concourse.mybir as mybir
from concourse import bass_utils

N = 1024
EMB = 128
HID = 128
NCORES = 8
SH = N // NCORES  # i-rows per core
F32 = mybir.dt.float32
F32R = mybir.dt.float32r  # fp32 bits, single-pass reduced-precision matmul
BF16 = mybir.dt.bfloat16
GELU = mybir.ActivationFunctionType.Gelu


def _build():
    nc = bacc.Bacc("TRN2", target_bir_lowering=False, debug=False)

    z1Ta_d = nc.dram_tensor("z1Ta", (EMB, 512), F32, kind="ExternalInput")
    z1Tb_d = nc.dram_tensor("z1Tb", (EMB, 512), F32, kind="ExternalInput")
    z2T_d = nc.dram_tensor("z2T", (EMB, SH), F32, kind="ExternalInput")
    w1aT_d = nc.dram_tensor("w1aT", (EMB, HID), F32, kind="ExternalInput")
    w1bT_d = nc.dram_tensor("w1bT", (EMB, HID), F32, kind="ExternalInput")
    w2T_d = nc.dram_tensor("w2T", (HID, HID), F32, kind="ExternalInput")
    w3p_d = nc.dram_tensor("w3p", (HID, 32), F32, kind="ExternalInput")
    b1_d = nc.dram_tensor("b1", (HID,), F32, kind="ExternalInput")
    b2_d = nc.dram_tensor("b2", (HID,), F32, kind="ExternalInput")
    b3_d = nc.dram_tensor("b3", (1,), F32, kind="ExternalInput")
    out_d = nc.dram_tensor("out", (SH, N), F32, kind="ExternalOutput")

    with tile.TileContext(nc) as tc:
        _body(
            tc, out_d, z1Ta_d, z1Tb_d, z2T_d, w1aT_d, w1bT_d, w2T_d, w3p_d,
            b1_d, b2_d, b3_d,
        )

    nc.compile()
    return nc


def _body(tc, out_d, z1Ta_d, z1Tb_d, z2T_d, w1aT_d, w1bT_d, w2T_d, w3p_d, b1_d, b2_d, b3_d):
    nc = tc.nc
    with (
        tc.tile_pool(name="const", bufs=1) as const,
        tc.tile_pool(name="h1p", bufs=4) as h1p,
        tc.tile_pool(name="h2p", bufs=6) as h2p,
        tc.tile_pool(name="srows", bufs=6) as srows,
        tc.tile_pool(name="ps", bufs=2, space="PSUM") as ps,
        tc.tile_pool(name="apz", bufs=1, space="PSUM") as apz,
        tc.tile_pool(name="rowp", bufs=1, space="PSUM") as rowp,
    ):
        # Dummy 1-element gelu so the ~2.7us ACT table load for the gelu set
        # runs at t=0, off the critical path of the first real gelu.
        tiny = const.tile([1, 1], F32)
        nc.vector.memset(tiny, 0.0)
        warm = const.tile([1, 1], F32)
        nc.scalar.activation(warm, tiny, GELU)

        # ---- load inputs (z1T split across two DMA queues) ----
        z1T_sb = const.tile([128, N], F32)
        nc.sync.dma_start(out=z1T_sb[:, 0:512], in_=z1Ta_d.ap())
        nc.gpsimd.dma_start(out=z1T_sb[:, 512:1024], in_=z1Tb_d.ap())
        z2T_sb = const.tile([128, SH], F32)
        nc.gpsimd.dma_start(out=z2T_sb, in_=z2T_d.ap())
        w1aT_sb = const.tile([128, HID], F32)
        nc.sync.dma_start(out=w1aT_sb, in_=w1aT_d.ap())
        w1bT_sb = const.tile([128, HID], F32)
        nc.gpsimd.dma_start(out=w1bT_sb, in_=w1bT_d.ap())
        w2T_f = const.tile([128, HID], F32)
        nc.sync.dma_start(out=w2T_f, in_=w2T_d.ap())
        w3p_f = const.tile([128, 32], F32)
        nc.sync.dma_start(out=w3p_f, in_=w3p_d.ap())
        b1_sb = const.tile([128, 1], F32)
        nc.gpsimd.dma_start(out=b1_sb, in_=b1_d.ap().rearrange("(p o) -> p o", o=1))
        b2_sb = const.tile([128, 1], F32)
        nc.sync.dma_start(out=b2_sb, in_=b2_d.ap().rearrange("(p o) -> p o", o=1))
        b3_sb = const.tile([1, 1], F32)
        nc.sync.dma_start(out=b3_sb, in_=b3_d.ap().rearrange("(p o) -> p o", o=1))

        # fp32r / bf16 lhsT casts (DVE rounds on write, as the fp32r matmul
        # consumer requires)
        w2T_sb = const.tile([128, HID], F32R)
        nc.vector.tensor_copy(w2T_sb, w2T_f)
        w3T_sb = const.tile([128, 32], BF16)
        nc.vector.tensor_copy(w3T_sb, w3p_f)

        # ---- a[h, j] for all j (PSUM-resident); b_pp[h, i] = b[h, i] + b1 ----
        a_ps = apz.tile([128, N], F32)
        nc.tensor.matmul(a_ps[:, 0:512], w1aT_sb, z1T_sb[:, 0:512])
        nc.tensor.matmul(a_ps[:, 512:1024], w1aT_sb, z1T_sb[:, 512:1024])

        tpb = ps.tile([128, 1024], F32, tag="ps1")
        nc.tensor.matmul(tpb[:, 0:SH], w1bT_sb, z2T_sb)
        b_pp_sb = const.tile([128, SH], F32)
        nc.vector.tensor_scalar_add(b_pp_sb, tpb[:, 0:SH], b1_sb[:, 0:1])

        # ---- main loop over my 128 i values ----
        # Software-pipelined emission: gelu1 (per-i bias via the ACT bias
        # port) runs 2 iterations ahead so the ACT stream never stalls on the
        # W2 matmuls. The W3-dot runs as 4 concurrent bf16 M=32 column-tile
        # matmuls (outputs at PSUM partitions 0/32/64/96) per 4 i's.
        h1s = [None] * SH

        def emit_g1(i):
            h1s[i] = h1p.tile([128, N], F32R, tag="h1", name="h1")
            nc.scalar.activation(h1s[i], a_ps, GELU, bias=b_pp_sb[:, i : i + 1])

        emit_g1(0)
        emit_g1(1)
        h2s = [None] * 4
        for i in range(SH):
            if i + 2 < SH:
                emit_g1(i + 2)

            ps1 = ps.tile([128, N], F32, tag="ps1")
            nc.tensor.matmul(ps1[:, 0:512], w2T_sb, h1s[i][:, 0:512])
            nc.tensor.matmul(ps1[:, 512:1024], w2T_sb, h1s[i][:, 512:1024])
            h1s[i] = None

            h2 = h2p.tile([128, N], BF16, tag="h2")
            nc.scalar.activation(h2, ps1, GELU, bias=b2_sb[:, 0:1])
            h2s[i % 4] = h2

            if i % 4 == 3:
                pr = rowp.tile([128, N], F32, tag="row")
                for k in range(4):
                    nc.tensor.matmul(
                        pr[32 * k : 32 * k + 32, 0:512],
                        w3T_sb,
                        h2s[k][:, 0:512],
                        tile_position=(0, 32 * k),
                    )
                    nc.tensor.matmul(
                        pr[32 * k : 32 * k + 32, 512:1024],
                        w3T_sb,
                        h2s[k][:, 512:1024],
                        tile_position=(0, 32 * k),
                    )
                for k in range(4):
                    ii = i - 3 + k
                    srow = srows.tile([1, N], F32, tag="srow")
                    nc.vector.tensor_scalar_add(
                        srow, pr[32 * k : 32 * k + 1, :], b3_sb[0:1, 0:1]
                    )
                    nc.sync.dma_start(out=out_d.ap()[ii : ii + 1, :], in_=srow)


_NC_CACHE = None


def make_in_maps(z1, z2, W1, b1, W2, b2, W3, b3):
    f = np.float32
    z1 = np.asarray(z1, dtype=f)
    z2 = np.asarray(z2, dtype=f)
    W1 = np.asarray(W1, dtype=f)
    b1 = np.ascontiguousarray(np.asarray(b1, dtype=f))
    W2 = np.asarray(W2, dtype=f)
    b2 = np.ascontiguousarray(np.asarray(b2, dtype=f))
    W3 = np.asarray(W3, dtype=f)
    b3 = np.ascontiguousarray(np.asarray(b3, dtype=f))

    # Host-side relayout only (no math): transposes, the i-shard split of
    # z2, and zero-padding W3 to an M=32 column tile.
    z1T = np.ascontiguousarray(z1.T)
    z1Ta = np.ascontiguousarray(z1T[:, 0:512])
    z1Tb = np.ascontiguousarray(z1T[:, 512:1024])
    w1aT = np.ascontiguousarray(W1[:, :EMB].T)
    w1bT = np.ascontiguousarray(W1[:, EMB:].T)
    w2T = np.ascontiguousarray(W2.T)
    w3p = np.zeros((HID, 32), dtype=f)
    w3p[:, 0] = W3[0]

    return [
        {
            "z1Ta": z1Ta,
            "z1Tb": z1Tb,
            "z2T": np.ascontiguousarray(z2[c * SH : (c + 1) * SH].T),
            "w1aT": w1aT,
            "w1bT": w1bT,
            "w2T": w2T,
            "w3p": w3p,
            "b1": b1,
            "b2": b2,
            "b3": b3,
        }
        for c in range(NCORES)
    ]


def kernel(z1, z2, W1, b1, W2, b2, W3, b3):
    global _NC_CACHE
    if _NC_CACHE is None:
        _NC_CACHE = _build()
    nc = _NC_CACHE

    in_maps = make_in_maps(z1, z2, W1, b1, W2, b2, W3, b3)
    res = bass_utils.run_bass_kernel_spmd(nc, in_maps, core_ids=list(range(NCORES)))
    return np.concatenate([r["out"] for r in res.results], axis=0)


if __name__ == "__main__":
    rng = np.random.default_rng(0)
    s1 = 1.0 / np.sqrt(2 * EMB)
    s2 = 1.0 / np.sqrt(HID)
    ins = dict(
        z1=rng.standard_normal((N, EMB), dtype=np.float32),
        z2=rng.standard_normal((N, EMB), dtype=np.float32),
        W1=rng.uniform(-s1, s1, (HID, 2 * EMB)).astype(np.float32),
        b1=rng.uniform(-s1, s1, (HID,)).astype(np.float32),
        W2=rng.uniform(-s2, s2, (HID, HID)).astype(np.float32),
        b2=rng.uniform(-s2, s2, (HID,)).astype(np.float32),
        W3=rng.uniform(-s2, s2, (1, HID)).astype(np.float32),
        b3=rng.uniform(-s2, s2, (1,)).astype(np.float32),
    )
    out = kernel(**ins)
    print("out", out.shape, out.dtype, out[:2, :4])
